# revision 1
# baseline (speedup 1.0000x reference)
"""BNAF layer kernel for 8x Trainium2 NeuronCores (Bass/Tile).

Math (per sample s = (b, w)):
    h_w = tanh(w_w1 @ e + w_b1)                  [256]
    w1  = (w_w2 @ h_w + w_b2) -> [I=64, O=64]
    h_b = tanh(b_w1 @ e + b_b1)                  [256]
    b1  = b_w2 @ h_b + b_b2                      [64]
    out[o]  = sum_i input[i] * exp(w1[i,o]) + b1[o]
    lj[o]   = logsumexp_i(w1[i,o] + logj[i])

Reformulation used here:
    W1a[s, f'] = w1[s,i,o] + logj[s,i] + 0      (f' = o*64+i, o-major)
  computed as ONE augmented GEMM with K = 256 + 64 + 1 = 321:
    H_aug = [tanh(GEMM1) (256) | logj (64) | ones (1)]
    W2aug = [w_w2^T        | Sel(i)    | w_b2 ]
  Then with P2 = exp(W1a):
    lj[s,o]  = log(sum_i P2[s, o*64+i])
    out[s,o] = sum_i g[s,i] * P2[s, o*64+i] + b1[s,o],
  where g = input * exp(-logj) exactly cancels the folded logj.
  (g is computed host-side against the bf16-rounded logj so the
  cancellation is exact.)

Sharding: data-parallel over B across the 8 cores (32 b-rows each),
weights replicated. No collectives.
"""

import os
import sys

import numpy as np

# ---- problem constants (hardcoded; kernel.py must be self-contained) ----
B, W, IDIM, ODIM, WIN = 256, 64, 64, 64, 128
H2 = 2 * WIN            # 256 hidden
F = IDIM * ODIM         # 4096
NCORES = 8
BS = B // NCORES        # 32 b-rows per core
NS = BS * W             # 2048 samples per core
ST = 128                # samples per tile (partition dim)
NT = NS // ST           # 16 tiles
KAUG = H2 + IDIM + 1    # 321

_PROG = None  # cached compiled program


def _ensure_path():
    for p in ("/opt/trn_rl_repo",):
        if p not in sys.path:
            sys.path.insert(0, p)


def _build_program(use_biases=True):
    """Build + schedule + compile the (SPMD, per-core) Bass program."""
    _ensure_path()
    import concourse.bass as bass
    import concourse.tile as tile
    from concourse import bacc, mybir

    f32 = mybir.dt.float32
    bf16 = mybir.dt.bfloat16
    AF = mybir.ActivationFunctionType
    ALU = mybir.AluOpType
    AX = mybir.AxisListType

    nc = bacc.Bacc("TRN2", target_bir_lowering=False, debug=False,
                   num_devices=NCORES)

    # -------- DRAM tensors (per-core inputs) --------
    # packed per-tile inputs: [:, :, 0:128]=embT-slice (e on rows),
    # [:, :, 128:192]=g rows, [:, 0:65, 192:320]=[logjT; ones] block
    d_xin = nc.dram_tensor("xin", [NT, 128, 320], bf16,
                           kind="ExternalInput")
    d_w2aug = nc.dram_tensor("w2aug", [KAUG, F], bf16, kind="ExternalInput")
    d_bnet = nc.dram_tensor("bnet", [KAUG, ODIM], bf16, kind="ExternalInput")
    d_w1T = nc.dram_tensor("w1T", [WIN, H2], bf16, kind="ExternalInput")
    d_b1T = nc.dram_tensor("b1T", [WIN, H2], bf16, kind="ExternalInput")
    d_wb1 = nc.dram_tensor("wb1", [H2, 1], f32, kind="ExternalInput")
    d_bb1 = nc.dram_tensor("bb1", [H2, 1], f32, kind="ExternalInput")
    d_out = nc.dram_tensor("out", [NS, ODIM], f32, kind="ExternalOutput")
    d_lj = nc.dram_tensor("lj", [NS, ODIM], f32, kind="ExternalOutput")

    mul_on_gp = bool(os.environ.get("BNAF_MUL_GP"))
    repeat = int(os.environ.get("BNAF_REPEAT", "1"))

    with tile.TileContext(nc) as tc:
        from contextlib import ExitStack
        with ExitStack() as ctx:
            singles = ctx.enter_context(tc.tile_pool(name="singles", bufs=1))
            work = ctx.enter_context(tc.tile_pool(name="work", bufs=3))
            psg2 = ctx.enter_context(
                tc.tile_pool(name="psg2", bufs=3, space="PSUM"))
            psmisc = ctx.enter_context(
                tc.tile_pool(name="psmisc", bufs=2, space="PSUM"))

            # ---- static weights into SBUF ----
            w1T = singles.tile([WIN, H2], bf16, tag="w1T")
            b1T = singles.tile([WIN, H2], bf16, tag="b1T")
            if use_biases:
                wb1 = singles.tile([128, 2], f32, tag="wb1")
                bb1 = singles.tile([128, 2], f32, tag="bb1")
                nc.sync.dma_start(out=wb1[:, 0:1], in_=d_wb1[0:128, :])
                nc.sync.dma_start(out=wb1[:, 1:2], in_=d_wb1[128:256, :])
                nc.sync.dma_start(out=bb1[:, 0:1], in_=d_bb1[0:128, :])
                nc.sync.dma_start(out=bb1[:, 1:2], in_=d_bb1[128:256, :])
            w2_c1 = singles.tile([128, F], bf16, tag="w2c1")
            w2_c2 = singles.tile([128, F], bf16, tag="w2c2")
            w2_c3 = singles.tile([KAUG - 256, F], bf16, tag="w2c3")
            nc.sync.dma_start(out=w2_c1, in_=d_w2aug[0:128, :])
            nc.sync.dma_start(out=w1T, in_=d_w1T[:, :])
            nc.sync.dma_start(out=b1T, in_=d_b1T[:, :])
            nc.sync.dma_start(out=w2_c2, in_=d_w2aug[128:256, :])
            nc.sync.dma_start(out=w2_c3, in_=d_w2aug[256:KAUG, :])
            bn_c1 = singles.tile([128, ODIM], bf16, tag="bnc1")
            bn_c2 = singles.tile([128, ODIM], bf16, tag="bnc2")
            bn_c3 = singles.tile([KAUG - 256, ODIM], bf16, tag="bnc3")
            nc.sync.dma_start(out=bn_c1, in_=d_bnet[0:128, :])
            nc.sync.dma_start(out=bn_c2, in_=d_bnet[128:256, :])
            nc.sync.dma_start(out=bn_c3, in_=d_bnet[256:KAUG, :])
            accAB_g = [singles.tile([128, 512], f32, tag=f"accABg{gi}",
                                      name=f"accAB_g{gi}") for gi in range(4)]
            out_g = [singles.tile([128, 4, ODIM], f32, tag=f"outg{gi}",
                                  name=f"out_g{gi}") for gi in range(4)]

            # ======== per-tile pipeline ========
            for ti in range(repeat * NT):
                t = ti % NT
                s0 = t * ST
                sl = slice(s0, s0 + ST)

                X = work.tile([128, 320], bf16, tag="X", name=f"X_{ti}",
                              bufs=3)
                nc.scalar.dma_start(out=X, in_=d_xin[t])
                et = X[:, 0:128]
                gt = X[:, 128:192]
                c3 = X[0:KAUG - 256, 192:320]

                # GEMM1 (both hypernets): [h, s] column blocks
                h_ps = psmisc.tile([128, 512], f32, tag="ps",
                                   name=f"hps_{ti}")
                for j, (lhs, hs) in enumerate((
                        (w1T, slice(0, 128)), (w1T, slice(128, 256)),
                        (b1T, slice(0, 128)), (b1T, slice(128, 256)))):
                    nc.tensor.matmul(
                        h_ps[:, j * 128:(j + 1) * 128],
                        lhs[:, hs], et, start=True, stop=True)
                Ht = work.tile([128, 512], bf16, tag="H", name=f"H_{ti}",
                               bufs=3)
                if use_biases:
                    for j, (bias, col) in enumerate((
                            (wb1, 0), (wb1, 1), (bb1, 0), (bb1, 1))):
                        nc.scalar.activation(
                            Ht[:, j * 128:(j + 1) * 128],
                            h_ps[:, j * 128:(j + 1) * 128],
                            AF.Tanh, bias=bias[:, col:col + 1])
                else:
                    nc.scalar.activation(Ht, h_ps, AF.Tanh)

                # b-net output head: b1[s, o]
                b_ps = psmisc.tile([128, ODIM], f32, tag="ps",
                                   name=f"bps_{ti}")
                for k, lhs in enumerate((Ht[:, 256:384], Ht[:, 384:512], c3)):
                    nc.tensor.matmul(
                        b_ps, lhs, (bn_c1, bn_c2, bn_c3)[k],
                        start=(k == 0), stop=(k == 2))
                b1 = work.tile([128, ODIM], f32, tag="b1", name=f"b1_{ti}",
                               bufs=4)
                nc.scalar.activation(b1, b_ps, AF.Copy)

                # GEMM2 augmented + exp; MP = [M(4096) | P2(4096)]
                MP = work.tile([128, 2 * F], bf16, tag="MP", name=f"MP_{ti}",
                                bufs=4)
                P2 = MP[:, F:2 * F]
                for grp in range(2):
                    pss = [psg2.tile([128, 1024], f32, tag="g2",
                                     name=f"g2_{ti}_{grp}_{fi}")
                           for fi in range(2)]
                    lhss = (Ht[:, 0:128], Ht[:, 128:256], c3)
                    if ti == 0:
                        # fc-major for the first tile: finish each psum
                        # pair ASAP so exp/DVE start early
                        order = [(k, fi) for fi in range(4)
                                 for k in range(3)]
                    else:
                        order = [(k, fi) for k in range(3)
                                 for fi in range(4)]
                    for k, fi in order:
                        fc = grp * 4 + fi
                        rhs_t = (w2_c1, w2_c2, w2_c3)[k]
                        nc.tensor.matmul(
                            pss[fi // 2][:, (fi % 2) * 512:
                                         (fi % 2) * 512 + 512],
                            lhss[k],
                            rhs_t[:, fc * 512:(fc + 1) * 512],
                            start=(k == 0), stop=(k == 2))
                    for fi in range(2):
                        fc2 = grp * 2048 + fi * 1024
                        nc.scalar.activation(
                            P2[:, fc2:fc2 + 1024], pss[fi], AF.Exp)

                # weighted product M = g (bcast over o) * P2
                p2v = P2.rearrange("p (o i) -> p o i", i=IDIM)
                gbc = bass.AP(tensor=gt.tensor, offset=gt.offset,
                              ap=[list(gt.ap[0]), [0, ODIM], [1, IDIM]])
                mv = MP[:, 0:F].rearrange("p (o i) -> p o i", i=IDIM)
                eng = nc.gpsimd if mul_on_gp else nc.vector
                eng.tensor_tensor(out=mv, in0=p2v, in1=gbc, op=ALU.mult)

                # fused tree reduction over i for both halves (q = 128 pages)
                v = MP[:, :].rearrange("p (q i) -> p q i", i=IDIM)
                t1 = work.tile([128, F], bf16, tag="tr1", name=f"tr1_{ti}")
                v1 = t1[:, :].rearrange("p (q i) -> p q i", i=IDIM // 2)
                nc.vector.tensor_add(v1, v[:, :, 0:32], v[:, :, 32:64])
                t2 = work.tile([128, F // 2], bf16, tag="tr2",
                               name=f"tr2_{ti}")
                v2 = t2[:, :].rearrange("p (q i) -> p q i", i=IDIM // 4)
                nc.vector.tensor_add(v2, v1[:, :, 0:16], v1[:, :, 16:32])
                t3 = work.tile([128, F // 4], bf16, tag="tr3",
                               name=f"tr3_{ti}")
                v3 = t3[:, :].rearrange("p (q i) -> p q i", i=IDIM // 8)
                nc.vector.tensor_add(v3, v2[:, :, 0:8], v2[:, :, 8:16])
                t4 = work.tile([128, F // 8], bf16, tag="tr4",
                               name=f"tr4_{ti}")
                v4 = t4[:, :].rearrange("p (q i) -> p q i", i=4)
                nc.vector.tensor_add(v4, v3[:, :, 0:4], v3[:, :, 4:8])
                t5 = work.tile([128, F // 16], bf16, tag="tr5",
                               name=f"tr5_{ti}")
                v5 = t5[:, :].rearrange("p (q i) -> p q i", i=2)
                nc.vector.tensor_add(v5, v4[:, :, 0:2], v4[:, :, 2:4])
                acc_sl = accAB_g[t // 4][:, (t % 4) * 128:(t % 4 + 1) * 128]
                nc.vector.tensor_add(acc_sl, v5[:, :, 0:1][:, :, 0],
                                     v5[:, :, 1:2][:, :, 0])

                nc.vector.tensor_add(out_g[t // 4][:, t % 4, :],
                                     acc_sl[:, 0:ODIM], b1)
                if t % 4 == 3:
                    gi = t // 4
                    dst = d_out[gi * 4 * ST:(gi + 1) * 4 * ST, :].rearrange(
                        "(blk p) c -> p blk c", p=ST)
                    nc.sync.dma_start(out=dst, in_=out_g[gi])

            # ======== batched log + store (fenced to stay at the end,
            # so Ln's act-table swap happens once, not per tile) ========
            tc.no_sync_barrier()
            for gi in range(4):
                ljt = work.tile([128, 4, ODIM], f32, tag="ljt",
                                name=f"ljt_{gi}")
                nc.scalar.activation(
                    ljt, bass.AP(tensor=accAB_g[gi].tensor,
                                 offset=accAB_g[gi].offset + ODIM,
                                 ap=[accAB_g[gi].ap[0], [128, 4], [1, ODIM]]),
                    AF.Ln)
                dst = d_lj[gi * 4 * ST:(gi + 1) * 4 * ST, :].rearrange(
                    "(blk p) c -> p blk c", p=ST)
                nc.sync.dma_start(out=dst, in_=ljt)

    nc.compile()
    return nc


def _prep_inputs(inputs):
    """Host-side prep: weight transforms + per-core shards."""
    import ml_dtypes
    bf = ml_dtypes.bfloat16

    inp = np.asarray(inputs["input"], np.float32)
    emb = np.asarray(inputs["w_embeddings"], np.float32)
    logj = np.asarray(inputs["logj"], np.float32)
    w_w1 = np.asarray(inputs["w_w1"], np.float32)
    w_b1 = np.asarray(inputs["w_b1"], np.float32)
    w_w2 = np.asarray(inputs["w_w2"], np.float32)
    w_b2 = np.asarray(inputs["w_b2"], np.float32)
    b_w1 = np.asarray(inputs["b_w1"], np.float32)
    b_b1 = np.asarray(inputs["b_b1"], np.float32)
    b_w2 = np.asarray(inputs["b_w2"], np.float32)
    b_b2 = np.asarray(inputs["b_b2"], np.float32)

    # f' = o*64 + i  <->  f = i*64 + o
    fp = np.arange(F)
    i_ = fp % IDIM
    o_ = fp // IDIM
    old = i_ * ODIM + o_

    w2aug = np.zeros((KAUG, F), np.float32)
    w2aug[0:H2, :] = w_w2.T[:, old]
    w2aug[H2:H2 + IDIM, :] = (i_[None, :] == np.arange(IDIM)[:, None])
    w2aug[H2 + IDIM, :] = w_b2[old]

    bnet = np.zeros((KAUG, ODIM), np.float32)
    bnet[0:H2, :] = b_w2.T
    bnet[H2 + IDIM, :] = b_b2

    shared = {
        "w2aug": w2aug.astype(bf),
        "bnet": bnet.astype(bf),
        "w1T": w_w1.T.astype(bf).copy(),
        "b1T": b_w1.T.astype(bf).copy(),
        "wb1": w_b1.reshape(H2, 1).copy(),
        "bb1": b_b1.reshape(H2, 1).copy(),
    }

    in_maps = []
    for c in range(NCORES):
        bsl = slice(c * BS, (c + 1) * BS)
        emb_c = emb[bsl].reshape(NS, WIN)
        logj_c = logj[bsl].reshape(NS, IDIM)
        inp_c = inp[bsl].reshape(NS, IDIM)
        logj_bf = logj_c.astype(bf)
        # g computed against the bf16-rounded logj => exact cancellation
        g_c = inp_c * np.exp(-logj_bf.astype(np.float32))
        xin = np.zeros((NT, 128, 320), bf)
        # embT slice: rows = e, cols = s within tile
        xin[:, :, 0:WIN] = (emb_c.T.astype(bf)
                            .reshape(WIN, NT, ST).transpose(1, 0, 2))
        xin[:, :, WIN:WIN + IDIM] = g_c.astype(bf).reshape(NT, ST, IDIM)
        xin[:, 0:IDIM, WIN + IDIM:WIN + IDIM + ST] = (
            logj_bf.T.reshape(IDIM, NT, ST).transpose(1, 0, 2))
        xin[:, IDIM, WIN + IDIM:WIN + IDIM + ST] = 1.0
        in_maps.append({"xin": xin, **shared})
    return in_maps


def kernel(**inputs):
    global _PROG
    _ensure_path()
    in_maps = _prep_inputs(inputs)

    use_biases = any(
        np.any(np.asarray(inputs[k]) != 0)
        for k in ("w_b1", "b_b1"))
    if _PROG is None or _PROG[0] != use_biases:
        _PROG = (use_biases, _build_program(use_biases=use_biases))
    nc = _PROG[1]

    if os.environ.get("BNAF_SIM"):
        # single-core CoreSim validation path (core 0 only)
        from concourse.bass_interp import CoreSim
        sim = CoreSim(nc, trace=False)
        for k, v in in_maps[0].items():
            sim.tensor(k)[:] = v
        sim.simulate()
        res0 = {"out": np.array(sim.tensor("out")),
                "lj": np.array(sim.tensor("lj"))}
        results = [res0] * NCORES
    else:
        from concourse.bass_utils import run_bass_kernel_spmd
        trace = bool(os.environ.get("BNAF_TRACE"))
        r = run_bass_kernel_spmd(nc, in_maps, core_ids=list(range(NCORES)),
                                 trace=trace)
        if trace:
            print(f"HW exec time: {r.exec_time_ns} ns "
                  f"(mean {r.mean_exec_time_ns})")
            if r.instructions_and_trace:
                print("trace path:", r.instructions_and_trace[1])
        results = r.results

    out = np.empty((B, W, ODIM), np.float32)
    lj = np.empty((B, W, ODIM), np.float32)
    for c in range(NCORES):
        bsl = slice(c * BS, (c + 1) * BS)
        out[bsl] = np.asarray(results[c]["out"], np.float32).reshape(BS, W, ODIM)
        lj[bsl] = np.asarray(results[c]["lj"], np.float32).reshape(BS, W, ODIM)
    return (out, lj)

